# revision 3
# baseline (speedup 1.0000x reference)
"""Trainium2 Bass kernel for nn_BitGatConv_48524540510805.

Reference computation:
    nhs    = nodes_ft @ weight                      # [n, HC]
    f1     = nhs @ conv_weight1
    f2     = nhs @ conv_weight2
    logits = f1[:, None, :] + f2[None, :, :]        # [n, n, HC]
    scores = leaky_relu(logits) + adj_bias[:, :, None]
    coefs  = softmax(scores, axis=1)                # over source-node dim j
    vals   = sum_j coefs[i, j, c] * nhs[i, c]       # NOTE: nhs[i], not nhs[j]

Because the softmax normalizes over j and the weighted value is nhs[i, c]
(independent of j), the weighted sum telescopes:

    vals[i, c] = nhs[i, c] * sum_j coefs[i, j, c] = nhs[i, c]

(softmax rows always sum to 1; the mask bias is finite so no NaNs, and the
adjacency always includes self-loops anyway).  Verified numerically against
the full reference: max per-element relative error ~8e-7 (pure fp32
rounding of the softmax row-sum).  So the kernel only needs
nhs = nodes_ft @ weight.

Distribution: destination-node rows are sharded across the 8 cores
(192 rows each, per the sharding hint).  Each core computes
outT_c = weight.T @ x_c.T on the tensor engine.

Per-core device program (latency-optimized; all engine/DMA choices were
driven by TimelineSim cost-model iteration):
  - host packs xw_c = [x_c.T | weight] as one [256, 256] f32 array so each
    128-partition k-chunk (row-plane) of the packed tensor holds exactly the
    operands of one accumulating matmul;
  - plane A (k=0:128) loads via the SP HWDGE queue, plane B (k=128:256) in
    parallel via the gpsimd SWDGE queue;
  - PE runs matmul A as soon as plane A lands, accumulates matmul B on top
    in PSUM;
  - the Activation engine copies PSUM->SBUF and issues the output DMA, so
    the DMA descriptor generation overlaps the copy.
"""

import numpy as np

import concourse.bass as bass
import concourse.mybir as mybir
from concourse.bass_utils import run_bass_kernel_spmd

N = 1536
IN_CH = 256
HC = 64
N_CORES = 8
ROWS = N // N_CORES  # 192 destination rows per core
PK = ROWS + HC  # 256 packed columns: [x_c.T | w]

_FP32 = mybir.dt.float32

_built = None


def _build_bass(mm_dtype=_FP32):
    """Per-core program: outT[HC, ROWS] = w.T @ x_c.T, K=256 split in two."""
    nc = bass.Bass()
    xw = nc.dram_tensor("xw", [IN_CH, PK], mm_dtype, kind="ExternalInput")
    outT = nc.dram_tensor("outT", [HC, ROWS], _FP32, kind="ExternalOutput")

    with (
        nc.sbuf_tensor("tA", [128, PK], mm_dtype) as tA,
        nc.sbuf_tensor("tB", [128, PK], mm_dtype) as tB,
        nc.psum_tensor("ps", [HC, ROWS], _FP32) as ps,
        nc.sbuf_tensor("o", [HC, ROWS], _FP32) as o,
        nc.semaphore("dma_sem") as dma_sem,
        nc.semaphore("dmb_sem") as dmb_sem,
        nc.semaphore("pe_sem") as pe_sem,
        nc.Block() as block,
    ):

        @block.sync
        def _(sync):
            sync.dma_start(out=tA[:, :], in_=xw[0:128, :]).then_inc(dma_sem, 16)

        @block.gpsimd
        def _(gpsimd):
            gpsimd.dma_start(out=tB[:, :], in_=xw[128:256, :]).then_inc(dmb_sem, 16)

        @block.tensor
        def _(tensor):
            tensor.wait_ge(dma_sem, 16)
            tensor.matmul(ps[:, :], tA[:, ROWS:PK], tA[:, 0:ROWS], start=True, stop=False)
            tensor.wait_ge(dmb_sem, 16)
            tensor.matmul(ps[:, :], tB[:, ROWS:PK], tB[:, 0:ROWS], start=False, stop=True).then_inc(pe_sem, 1)

        @block.scalar
        def _(scalar):
            scalar.wait_ge(pe_sem, 1)
            scalar.copy(o[:, :], ps[:, :])
            scalar.dma_start(out=outT[:, :], in_=o[:, :]).then_inc(dma_sem, 16)

    return nc


def _shard_inputs(nodes_ft, weight):
    nodes_ft = np.ascontiguousarray(nodes_ft, dtype=np.float32)
    w = np.ascontiguousarray(weight, dtype=np.float32)
    in_maps = []
    for c in range(N_CORES):
        x_c = nodes_ft[c * ROWS : (c + 1) * ROWS, :]  # [ROWS, IN_CH]
        xw = np.empty((IN_CH, PK), dtype=np.float32)
        xw[:, 0:ROWS] = x_c.T
        xw[:, ROWS:PK] = w
        in_maps.append({"xw": xw})
    return in_maps


def kernel(nodes_ft, adj_bias_mat, weight, conv_weight1, conv_weight2):
    global _built
    if _built is None:
        _built = _build_bass()

    in_maps = _shard_inputs(nodes_ft, weight)
    res = run_bass_kernel_spmd(_built, in_maps, list(range(N_CORES)))

    out = np.empty((N, HC), dtype=np.float32)
    for c in range(N_CORES):
        out[c * ROWS : (c + 1) * ROWS, :] = res.results[c]["outT"].T
    return out
